# revision 18
# baseline (speedup 1.0000x reference)
"""Trainium2 Bass kernel for nn_BasicBlockShared (MoE-routed residual block).

Reference computation (per sample b):
    r = sigmoid(GAP(x) @ router_w.T + router_b)          # [B, E]
    k1 = sum_e r[b,e] * w1[e]                            # per-sample conv kernel
    y1 = relu(bn1(conv3x3(x[b], k1)))
    k2 = sum_e r[b,e] * w2[e]
    out = relu(bn2(conv3x3(y1, k2)) + x[b])

Sharding: data-parallel over batch. 32 samples -> 4 per core x 8 cores.
Expert banks + router + bn params replicated on every core.

v3 pipeline: 4 og-phases (li0og0, li0og1, li1og0, li1og1), each split in
two row-half units (16 output rows x 4 samples, one PSUM bank per
sample).  The expert combination for phase p+1 runs interleaved inside
phase p's second (pure-conv) unit, so conv never waits on kT evictions;
phase 0 pipelines its own combination with a one-group lookahead.  Comb
group = one (og,dy,dx): 16 matmuls of 64 cols into a [128,1024] psum,
evicted to kT split across ScalarE (ig0) and DVE (ig1).  GAP reduces are
split DVE/gpsimd, the router matmul accumulates the four partials.
Output is written bf16 and cast back to f32 on host.
"""

import numpy as np
from contextlib import ExitStack

from concourse import bacc, mybir, tile
import concourse.bass as bass
from concourse.bass_utils import run_bass_kernel_spmd

B, C, H, W, E = 32, 256, 32, 32, 8
NCORES = 8
BS = B // NCORES            # samples per core
NG = C // 128               # channel groups (2)
NGRP = 18                   # (og, dy, dx) groups per layer
GCOLS = 16 * 128            # cols per group tile: 16 chunks x 128 i
PAD = H + 2                 # 34
EPS = 1e-5
BF = mybir.dt.bfloat16
F32 = mybir.dt.float32

_BUILT = {}


def build():
    nc = bacc.Bacc("TRN2", target_bir_lowering=False, debug=False,
                   num_devices=NCORES)
    x_d = nc.dram_tensor("x", [BS, C, H, W], BF, kind="ExternalInput")
    cp_d = nc.dram_tensor("cpack", [128, 32], F32, kind="ExternalInput")
    rb_d = nc.dram_tensor("router_b", [E], F32, kind="ExternalInput")
    w_d = [nc.dram_tensor("w1t", [128, NGRP * GCOLS], BF,
                          kind="ExternalInput"),
           nc.dram_tensor("w2t", [128, NGRP * GCOLS], BF,
                          kind="ExternalInput")]
    m16_d = nc.dram_tensor("mask16", [128, 16], BF, kind="ExternalInput")
    irep_d = nc.dram_tensor("irep", [E, 128], F32, kind="ExternalInput")
    out_d = nc.dram_tensor("out", [BS, C, H, W], BF, kind="ExternalOutput")

    with tile.TileContext(nc) as tc, ExitStack() as ctx:
        const = ctx.enter_context(tc.tile_pool(name="const", bufs=1))
        xpool = ctx.enter_context(tc.tile_pool(name="xpool", bufs=1))
        kpool = ctx.enter_context(tc.tile_pool(name="kpool", bufs=1))
        wpool = ctx.enter_context(tc.tile_pool(name="wpool", bufs=18))
        opool = ctx.enter_context(tc.tile_pool(name="opool", bufs=8))
        cpsum = ctx.enter_context(tc.tile_pool(name="cpsum", bufs=4, space="PSUM"))
        mpsum = ctx.enter_context(tc.tile_pool(name="mpsum", bufs=2, space="PSUM"))

        # ---- tiny consts on the scalar DMA ring; then the dummy sigmoid
        #      preloads the act table before the real (critical) sigmoid
        cp_sb = const.tile([128, 32], F32, tag="cpack")
        nc.scalar.dma_start(out=cp_sb, in_=bass.AP(tensor=cp_d, offset=0,
                                                   ap=[[32, 128], [1, 32]]))
        m16_sb = const.tile([128, 16], BF, tag="m16")
        nc.scalar.dma_start(out=m16_sb, in_=bass.AP(tensor=m16_d, offset=0,
                                                    ap=[[16, 128], [1, 16]]))
        irep_sb = const.tile([E, 128], F32, tag="irep")
        nc.scalar.dma_start(out=irep_sb, in_=bass.AP(tensor=irep_d, offset=0,
                                                     ap=[[128, E], [1, 128]]))
        rb_sb = const.tile([E, 1], F32, tag="rb")
        nc.scalar.dma_start(out=rb_sb,
                            in_=bass.AP(tensor=rb_d, offset=0,
                                        ap=[[1, E], [1, 1]]))
        eps_sb = const.tile([128, 1], F32, tag="eps")
        nc.vector.memset(eps_sb, EPS)
        dmy = const.tile([128, 1], F32, tag="dmy")
        nc.scalar.activation(out=dmy, in_=eps_sb,
                             func=mybir.ActivationFunctionType.Sigmoid,
                             bias=0.0, scale=1.0)
        nc.scalar.activation(out=dmy, in_=eps_sb,
                             func=mybir.ActivationFunctionType.Relu,
                             bias=0.0, scale=1.0)

        # ---- x staging, split across gpsimd+sync rings
        xs = [[xpool.tile([128, H * W], BF, tag=f"xs_{b}_{g}",
                          name=f"xs_{b}_{g}")
               for g in range(NG)] for b in range(BS)]
        xp = [[xpool.tile([128, PAD, PAD], BF, tag=f"xp_{b}_{g}",
                          name=f"xp_{b}_{g}")
               for g in range(NG)] for b in range(BS)]
        y1p = [[xpool.tile([128, PAD, PAD], BF, tag=f"y1p_{b}_{g}",
                           name=f"y1p_{b}_{g}")
                for g in range(NG)] for b in range(BS)]
        for b in range(BS):
            for g in range(NG):
                src = bass.AP(tensor=x_d,
                              offset=(b * C + g * 128) * H * W,
                              ap=[[H * W, 128], [1, H * W]])
                eng = nc.gpsimd if g == 0 else nc.sync
                eng.dma_start(out=xs[b][g], in_=src)

        # xp borders early on gpsimd (needed by first conv)
        for b in range(BS):
            for g in range(NG):
                nc.gpsimd.memset(xp[b][g][:, 0:PAD:33, :], 0.0)
                nc.gpsimd.memset(xp[b][g][:, 1:33, 0:PAD:33], 0.0)

        # ---- PE warm-up (independent single-shot matmuls) ----
        wu_in = const.tile([128, 512], BF, tag="wu_in")
        wu_w = const.tile([128, 128], BF, tag="wu_w")
        nc.vector.memset(wu_in, 0.0)
        nc.vector.memset(wu_w, 0.0)
        wu_ps = mpsum.tile([128, 1024], F32, tag="mps", name="wu_ps")
        NWU_A, NWU_B = 20, 2
        for wi in range(NWU_A):
            nc.tensor.matmul(wu_ps[:, 0:512], wu_w, wu_in,
                             start=True, stop=True)

        # ---- router: GAP reduces on DVE ----
        gap = [const.tile([128, BS], F32, tag=f"gap_{g}", name=f"gap_{g}")
               for g in range(NG)]
        for b in range(BS):
            for g in range(NG):
                nc.vector.tensor_reduce(out=gap[g][:, b:b + 1],
                                        in_=xs[b][g],
                                        axis=mybir.AxisListType.X,
                                        op=mybir.AluOpType.add)
        ps_rt = mpsum.tile([128, 1024], F32, tag="mps", name="ps_rt")
        ps_r = ps_rt[0:E, 0:BS]
        for g in range(NG):
            nc.tensor.matmul(ps_r, cp_sb[:, 16 + g * E:16 + (g + 1) * E],
                             gap[g], start=(g == 0), stop=(g == NG - 1))
        for wi in range(NWU_B):
            nc.tensor.matmul(wu_ps[:, 0:512], wu_w, wu_in,
                             start=True, stop=True)
        r_sb = const.tile([E, BS], F32, tag="r_sb")
        nc.scalar.activation(out=r_sb, in_=ps_r,
                             func=mybir.ActivationFunctionType.Sigmoid,
                             bias=rb_sb, scale=1.0 / (H * W))
        # re_bc[p, b] = r[p // 16, b] via PE broadcast
        re_t = mpsum.tile([128, 1024], F32, tag="mps", name="re_bc")
        re_bc = re_t[:, 0:BS]
        nc.tensor.matmul(re_bc, irep_sb, r_sb, start=True, stop=True)
        # R[(e,o16), (b,o16')] = r[e,b] * (o16 == o16') on DVE
        r_mat = const.tile([128, BS * 16], BF, tag="r_mat")
        for b in range(BS):
            nc.vector.tensor_scalar_mul(r_mat[:, b * 16:(b + 1) * 16],
                                        m16_sb, re_bc[:, b:b + 1])
        # pad-copies on gpsimd (independent of r_mat; DVE stays free
        # for the kT evictions)
        for b in range(BS):
            for g in range(NG):
                nc.gpsimd.tensor_scalar_mul(
                    xp[b][g][:, 1:33, 1:33],
                    xs[b][g].rearrange("p (r c) -> p r c", r=H), 1.0)

        # ---- bn scale/shift (after router: keeps Sqrt's table load off
        #      the sigmoid critical path; Sqrt on ScalarE, reciprocal on
        #      DVE, the muls on gpsimd) ----
        bn_sh = {}
        for li in range(2):
            g_sb = cp_sb[:, li * 8 + 0:li * 8 + 2]
            b_sb = cp_sb[:, li * 8 + 2:li * 8 + 4]
            m_sb = cp_sb[:, li * 8 + 4:li * 8 + 6]
            v_sb = cp_sb[:, li * 8 + 6:li * 8 + 8]
            sq = const.tile([128, NG], F32, tag=f"bn_sq{li}")
            nc.scalar.activation(out=sq, in_=v_sb,
                                 func=mybir.ActivationFunctionType.Sqrt,
                                 bias=eps_sb, scale=1.0)
            rs = const.tile([128, NG], F32, tag=f"bn_rs{li}")
            nc.vector.reciprocal(out=rs, in_=sq)
            s_sb = const.tile([128, NG], F32, tag=f"bn_s{li}")
            nc.gpsimd.tensor_mul(s_sb, g_sb, rs)
            t_sb = const.tile([128, NG], F32, tag=f"bn_t{li}")
            nc.gpsimd.tensor_mul(t_sb, m_sb, s_sb)
            h_sb = const.tile([128, NG], F32, tag=f"bn_h{li}")
            nc.gpsimd.tensor_sub(h_sb, b_sb, t_sb)
            bn_sh[li] = (s_sb, h_sb)

        # ---- bank group DMAs in consumption order: the first six on the
        #      sync ring (gpsimd's queue is busy with pads until ~16us),
        #      the rest alternating gpsimd/sync ----
        wt = {}
        for li in range(2):
            for gg in range(NGRP):
                k = li * NGRP + gg
                t = wpool.tile([128, GCOLS], BF, tag="wt",
                               name=f"wt_{li}_{gg}")
                eng = nc.sync if (k < 6 or k % 2 == 1) else nc.gpsimd
                eng.dma_start(
                    out=t,
                    in_=bass.AP(tensor=w_d[li], offset=gg * GCOLS,
                                ap=[[NGRP * GCOLS, 128], [1, GCOLS]]))
                wt[(li, gg)] = t

        # y1p borders (gpsimd; only needed before conv2 reads)
        for b in range(BS):
            for g in range(NG):
                nc.gpsimd.memset(y1p[b][g][:, 0:PAD:33, :], 0.0)
                nc.gpsimd.memset(y1p[b][g][:, 1:33, 0:PAD:33], 0.0)

        # ---- single kT buffer for both layers (range-tracked deps let
        #      layer-1 groups overwrite regions as layer-0 finishes) ----
        kT = kpool.tile([128, BS, NG * NGRP * 128], BF, tag="kT",
                        name="kT")

        def comb_group(li, gg):
            t = wt[(li, gg)]
            mps = mpsum.tile([128, 1024], F32, tag="mps",
                             name=f"mps_{li}_{gg}")
            for k in range(16):
                nc.tensor.matmul(mps[:, k * 64:(k + 1) * 64],
                                 t[:, k * 128:(k + 1) * 128], r_mat,
                                 start=True, stop=True)
            # evict: psum col = ((cc*2+ig)*4 + b)*16 + o  ->  kT
            src4 = mps.rearrange("p (cc ig b o) -> p ig b cc o",
                                 cc=8, ig=2, b=BS, o=16)
            for ig in range(NG):
                d0 = ig * NGRP * 128 + gg * 128
                dst = kT[:, :, d0:d0 + 128].rearrange(
                    "p b (cc o) -> p b cc o", cc=8, o=16)
                nc.scalar.activation(
                    out=dst, in_=src4[:, ig],
                    func=mybir.ActivationFunctionType.Copy,
                    bias=0.0, scale=1.0)

        def kslice(ig, gg, b):
            c0 = ig * NGRP * 128 + gg * 128
            return kT[:, b, c0:c0 + 128]

        def epilogue(li, og, rh, b, ps, osb):
            s_sb, h_sb = bn_sh[li]
            psr = ps.rearrange("p (r c) -> p r c", r=16)
            if li == 0:
                nc.scalar.activation(
                    out=y1p[b][og][:, 1 + rh * 16:1 + rh * 16 + 16, 1:33],
                    in_=psr,
                    func=mybir.ActivationFunctionType.Relu,
                    bias=h_sb[:, og:og + 1],
                    scale=s_sb[:, og:og + 1])
            else:
                nc.vector.scalar_tensor_tensor(
                    out=psr, in0=psr, scalar=s_sb[:, og:og + 1],
                    in1=xp[b][og][:, 1 + rh * 16:1 + rh * 16 + 16, 1:33],
                    op0=mybir.AluOpType.mult,
                    op1=mybir.AluOpType.add)
                nc.scalar.activation(
                    out=osb[b][:, rh * 512:rh * 512 + 512].rearrange(
                        "p (r c) -> p r c", r=16),
                    in_=psr,
                    func=mybir.ActivationFunctionType.Relu,
                    bias=h_sb[:, og:og + 1], scale=1.0)
                if rh == 1:
                    dst = bass.AP(
                        tensor=out_d,
                        offset=(b * C + og * 128) * H * W,
                        ap=[[H * W, 128], [1, H * W]])
                    eng = nc.gpsimd if b % 2 == 0 else nc.sync
                    eng.dma_start(out=dst, in_=osb[b])

        def conv_mm(li, og, rh, gl, ig, b, ps, src_t):
            gg = og * 9 + gl
            dy, dx = divmod(gl, 3)
            t = gl * 2 + ig
            nc.tensor.matmul(
                ps[b], kslice(ig, gg, b),
                src_t[b][ig][:, rh * 16 + dy:rh * 16 + dy + 16, dx:dx + 32],
                start=(t == 0), stop=(t == 17))

        def rh_unit(li, og, rh, osb=None, self_comb=False, pre_comb=None,
                    b_major=False):
            src_t = xp if li == 0 else y1p
            ps = [cpsum.tile([128, 512], F32, tag="cps",
                             name=f"cps_{li}_{og}_{rh}_{b}")
                  for b in range(BS)]
            if b_major:
                for b in range(BS):
                    for gl in range(9):
                        for ig in range(NG):
                            conv_mm(li, og, rh, gl, ig, b, ps, src_t)
                    epilogue(li, og, rh, b, ps[b], osb)
                return
            if self_comb:
                comb_group(li, og * 9)
                comb_group(li, og * 9 + 1)
            for gl in range(9):
                for ig in range(NG):
                    for b in range(BS):
                        conv_mm(li, og, rh, gl, ig, b, ps, src_t)
                        if gl == 8 and ig == NG - 1:
                            epilogue(li, og, rh, b, ps[b], osb)
                if ig == NG - 1 and self_comb and gl + 2 <= 8:
                    comb_group(li, og * 9 + gl + 2)
                if pre_comb is not None:
                    comb_group(*pre_comb[gl])

        # phase schedule; comb for phase p+1 inside phase p's rh1 unit
        osb_all = {}
        for og in range(2):
            osb_all[og] = [opool.tile([128, 1024], BF, tag="osb",
                                      name=f"osb_{og}_{b}")
                           for b in range(BS)]
        rh_unit(0, 0, 0, self_comb=True)
        rh_unit(0, 0, 1, pre_comb=[(0, 9 + gl) for gl in range(9)])
        rh_unit(0, 1, 0)
        rh_unit(0, 1, 1, pre_comb=[(1, gl) for gl in range(9)])
        rh_unit(1, 0, 0, osb=osb_all[0])
        rh_unit(1, 0, 1, osb=osb_all[0],
                pre_comb=[(1, 9 + gl) for gl in range(9)])
        rh_unit(1, 1, 0, osb=osb_all[1])
        rh_unit(1, 1, 1, osb=osb_all[1], b_major=True)
    nc.compile()
    return nc


def _get_nc():
    if "nc" not in _BUILT:
        _BUILT["nc"] = build()
    return _BUILT["nc"]


def _host_transform_bank(w):
    """[E, O, I, 3, 3] f32 -> [p=(e,o16), ((og,dy,dx), cc, ig, i)] bf16."""
    import ml_dtypes
    wr = w.reshape(E, NG, 128, NG, 128, 3, 3)      # e og o_l ig i_l dy dx
    t = wr.transpose(1, 5, 6, 3, 0, 2, 4)          # og dy dx ig e o_l i_l
    t = t.reshape(NGRP, NG, E, 8, 16, 128)         # gg ig e cc o16 i
    t = t.transpose(0, 3, 1, 2, 4, 5)              # gg cc ig e o16 i
    t = t.reshape(NGRP, 8, NG, 128, 128)           # gg cc ig p i
    t = t.transpose(3, 0, 1, 2, 4)                 # p gg cc ig i
    t = t.reshape(128, NGRP * GCOLS)
    return np.ascontiguousarray(t.astype(ml_dtypes.bfloat16))


def _host_mask16():
    import ml_dtypes
    m = (np.arange(128)[:, None] % 16 == np.arange(16)[None, :])
    return m.astype(ml_dtypes.bfloat16)


def _host_irep():
    return (np.arange(128)[None, :] // 16 ==
            np.arange(E)[:, None]).astype(np.float32)


def _host_cpack(f):
    """[128, 32] f32: cols (li*4+{g,b,m,v})*NG+g = bn, 16+g*E+e = router_w."""
    cp = np.zeros((128, 32), np.float32)
    for li, names in enumerate((("g1", "b1", "m1", "v1"),
                                ("g2", "b2", "m2", "v2"))):
        for k, nm in enumerate(names):
            cp[:, li * 8 + k * 2:li * 8 + k * 2 + 2] = \
                f[nm].reshape(NG, 128).T
    for g in range(NG):
        cp[:, 16 + g * E:16 + (g + 1) * E] = \
            f["router_w"][:, g * 128:(g + 1) * 128].T
    return cp


def run(inputs, trace=False):
    import ml_dtypes
    nc = _get_nc()
    full = {k: np.ascontiguousarray(np.asarray(v, dtype=np.float32))
            for k, v in inputs.items()}
    full["w1t"] = _host_transform_bank(full.pop("w1"))
    full["w2t"] = _host_transform_bank(full.pop("w2"))
    full["mask16"] = _host_mask16()
    full["irep"] = _host_irep()
    full["cpack"] = _host_cpack(full)
    for k in ("g1", "b1", "m1", "v1", "g2", "b2", "m2", "v2", "router_w"):
        full.pop(k)
    xbf = np.ascontiguousarray(full.pop("x").astype(ml_dtypes.bfloat16))
    in_maps = []
    for j in range(NCORES):
        m = dict(full)
        m["x"] = np.ascontiguousarray(xbf[j * BS:(j + 1) * BS])
        in_maps.append(m)
    res = run_bass_kernel_spmd(nc, in_maps, core_ids=list(range(NCORES)),
                               trace=trace)
    out = np.concatenate([res.results[j]["out"] for j in range(NCORES)],
                         axis=0).astype(np.float32)
    return out, res


def kernel(**inputs) -> np.ndarray:
    out, _ = run(inputs, trace=False)
    return out


# revision 22
# speedup vs baseline: 1.5779x; 1.5779x over previous
"""Trainium2 Bass kernel for nn_BasicBlockShared (MoE-routed residual block).

Reference computation (per sample b):
    r = sigmoid(GAP(x) @ router_w.T + router_b)          # [B, E]
    k1 = sum_e r[b,e] * w1[e]                            # per-sample conv kernel
    y1 = relu(bn1(conv3x3(x[b], k1)))
    k2 = sum_e r[b,e] * w2[e]
    out = relu(bn2(conv3x3(y1, k2)) + x[b])

Sharding: data-parallel over batch. 32 samples -> 4 per core x 8 cores.
Expert banks + router + bn params replicated on every core.

v3 pipeline: 4 og-phases (li0og0, li0og1, li1og0, li1og1), each split in
two row-half units (16 output rows x 4 samples, one PSUM bank per
sample).  The expert combination for phase p+1 runs interleaved inside
phase p's second (pure-conv) unit, so conv never waits on kT evictions;
phase 0 pipelines its own combination with a one-group lookahead.  Comb
group = one (og,dy,dx): 16 matmuls of 64 cols into a [128,1024] psum,
evicted to kT split across ScalarE (ig0) and DVE (ig1).  GAP reduces are
split DVE/gpsimd, the router matmul accumulates the four partials.
Output is written bf16 and cast back to f32 on host.
"""

import numpy as np
from contextlib import ExitStack

from concourse import bacc, mybir, tile
import concourse.bass as bass
from concourse.bass_utils import run_bass_kernel_spmd

B, C, H, W, E = 32, 256, 32, 32, 8
NCORES = 8
BS = B // NCORES            # samples per core
NG = C // 128               # channel groups (2)
NGRP = 18                   # (og, dy, dx) groups per layer
GCOLS = 16 * 128            # cols per group tile: 16 chunks x 128 i
PAD = H + 2                 # 34
EPS = 1e-5
BF = mybir.dt.bfloat16
F32 = mybir.dt.float32

_BUILT = {}


def build():
    nc = bacc.Bacc("TRN2", target_bir_lowering=False, debug=False,
                   num_devices=NCORES)
    x_d = nc.dram_tensor("x", [BS, C, H, W], BF, kind="ExternalInput")
    cp_d = nc.dram_tensor("cpack", [128, 32], F32, kind="ExternalInput")
    rb_d = nc.dram_tensor("router_b", [E], F32, kind="ExternalInput")
    w_d = [nc.dram_tensor("w1t", [128, NGRP * GCOLS], BF,
                          kind="ExternalInput"),
           nc.dram_tensor("w2t", [128, NGRP * GCOLS], BF,
                          kind="ExternalInput")]
    m16_d = nc.dram_tensor("mask16", [128, 16], BF, kind="ExternalInput")
    irep_d = nc.dram_tensor("irep", [E, 128], F32, kind="ExternalInput")
    out_d = nc.dram_tensor("out", [BS, C, H, W], BF, kind="ExternalOutput")

    with tile.TileContext(nc) as tc, ExitStack() as ctx:
        const = ctx.enter_context(tc.tile_pool(name="const", bufs=1))
        xpool = ctx.enter_context(tc.tile_pool(name="xpool", bufs=1))
        kpool = ctx.enter_context(tc.tile_pool(name="kpool", bufs=1))
        wpool = ctx.enter_context(tc.tile_pool(name="wpool", bufs=18))
        opool = ctx.enter_context(tc.tile_pool(name="opool", bufs=8))
        cpsum = ctx.enter_context(tc.tile_pool(name="cpsum", bufs=4, space="PSUM"))
        mpsum = ctx.enter_context(tc.tile_pool(name="mpsum", bufs=2, space="PSUM"))

        # ---- tiny consts on the scalar DMA ring; then the dummy sigmoid
        #      preloads the act table before the real (critical) sigmoid
        cp_sb = const.tile([128, 32], F32, tag="cpack")
        nc.scalar.dma_start(out=cp_sb, in_=bass.AP(tensor=cp_d, offset=0,
                                                   ap=[[32, 128], [1, 32]]))
        m16_sb = const.tile([128, 16], BF, tag="m16")
        nc.scalar.dma_start(out=m16_sb, in_=bass.AP(tensor=m16_d, offset=0,
                                                    ap=[[16, 128], [1, 16]]))
        irep_sb = const.tile([E, 128], F32, tag="irep")
        nc.scalar.dma_start(out=irep_sb, in_=bass.AP(tensor=irep_d, offset=0,
                                                     ap=[[128, E], [1, 128]]))
        rb_sb = const.tile([E, 1], F32, tag="rb")
        nc.scalar.dma_start(out=rb_sb,
                            in_=bass.AP(tensor=rb_d, offset=0,
                                        ap=[[1, E], [1, 1]]))
        eps_sb = const.tile([128, 1], F32, tag="eps")
        nc.vector.memset(eps_sb, EPS)
        dmy = const.tile([128, 1], F32, tag="dmy")
        nc.scalar.activation(out=dmy, in_=eps_sb,
                             func=mybir.ActivationFunctionType.Sigmoid,
                             bias=0.0, scale=1.0)
        nc.scalar.activation(out=dmy, in_=eps_sb,
                             func=mybir.ActivationFunctionType.Relu,
                             bias=0.0, scale=1.0)

        # ---- x staging, split across gpsimd+sync rings
        xs = [[xpool.tile([128, H * W], BF, tag=f"xs_{b}_{g}",
                          name=f"xs_{b}_{g}")
               for g in range(NG)] for b in range(BS)]
        xp = [[xpool.tile([128, PAD, PAD], BF, tag=f"xp_{b}_{g}",
                          name=f"xp_{b}_{g}")
               for g in range(NG)] for b in range(BS)]
        y1p = [[xpool.tile([128, PAD, PAD], BF, tag=f"y1p_{b}_{g}",
                           name=f"y1p_{b}_{g}")
                for g in range(NG)] for b in range(BS)]
        for b in range(BS):
            for g in range(NG):
                src = bass.AP(tensor=x_d,
                              offset=(b * C + g * 128) * H * W,
                              ap=[[H * W, 128], [1, H * W]])
                eng = nc.gpsimd if g == 0 else nc.sync
                eng.dma_start(out=xs[b][g], in_=src)

        # xp borders early on gpsimd (needed by first conv)
        for b in range(BS):
            for g in range(NG):
                nc.gpsimd.memset(xp[b][g][:, 0:PAD:33, :], 0.0)
                nc.gpsimd.memset(xp[b][g][:, 1:33, 0:PAD:33], 0.0)

        # ---- PE warm-up (independent single-shot matmuls) ----
        wu_in = const.tile([128, 512], BF, tag="wu_in")
        wu_w = const.tile([128, 128], BF, tag="wu_w")
        nc.vector.memset(wu_in, 0.0)
        nc.vector.memset(wu_w, 0.0)
        wu_ps = mpsum.tile([128, 1024], F32, tag="mps", name="wu_ps")
        NWU_A, NWU_B = 20, 2
        for wi in range(NWU_A):
            nc.tensor.matmul(wu_ps[:, 0:512], wu_w, wu_in,
                             start=True, stop=True)

        # ---- router: GAP reduces on DVE (tensor_tensor_reduce crashes
        #      the NRT on TRN2 hardware here -- do not use it) ----
        gap = [const.tile([128, BS], F32, tag=f"gap_{g}", name=f"gap_{g}")
               for g in range(NG)]
        for b in range(BS):
            for g in range(NG):
                nc.vector.tensor_reduce(out=gap[g][:, b:b + 1],
                                        in_=xs[b][g],
                                        axis=mybir.AxisListType.X,
                                        op=mybir.AluOpType.add)
        ps_rt = mpsum.tile([128, 1024], F32, tag="mps", name="ps_rt")
        ps_r = ps_rt[0:E, 0:BS]
        for g in range(NG):
            nc.tensor.matmul(ps_r, cp_sb[:, 16 + g * E:16 + (g + 1) * E],
                             gap[g], start=(g == 0), stop=(g == NG - 1))
        for wi in range(NWU_B):
            nc.tensor.matmul(wu_ps[:, 0:512], wu_w, wu_in,
                             start=True, stop=True)
        r_sb = const.tile([E, BS], F32, tag="r_sb")
        nc.scalar.activation(out=r_sb, in_=ps_r,
                             func=mybir.ActivationFunctionType.Sigmoid,
                             bias=rb_sb, scale=1.0 / (H * W))
        # re_bc[p, b] = r[p // 16, b] via PE broadcast
        re_t = mpsum.tile([128, 1024], F32, tag="mps", name="re_bc")
        re_bc = re_t[:, 0:BS]
        nc.tensor.matmul(re_bc, irep_sb, r_sb, start=True, stop=True)
        # R[(e,o16), (b,o16')] = r[e,b] * (o16 == o16') on DVE
        r_mat = const.tile([128, BS * 16], BF, tag="r_mat")
        for b in range(BS):
            nc.vector.tensor_scalar_mul(r_mat[:, b * 16:(b + 1) * 16],
                                        m16_sb, re_bc[:, b:b + 1])
        # pad-copies on DVE (gpsimd's TENSOR_SCALAR is a ~35x slower
        # ucode loop and starves concurrent DVE ops)
        for b in range(BS):
            for g in range(NG):
                nc.vector.tensor_scalar_mul(
                    xp[b][g][:, 1:33, 1:33],
                    xs[b][g].rearrange("p (r c) -> p r c", r=H), 1.0)

        # ---- bn scale/shift (after router: keeps Sqrt's table load off
        #      the sigmoid critical path; Sqrt on ScalarE, reciprocal on
        #      DVE, the muls on gpsimd) ----
        bn_sh = {}
        for li in range(2):
            g_sb = cp_sb[:, li * 8 + 0:li * 8 + 2]
            b_sb = cp_sb[:, li * 8 + 2:li * 8 + 4]
            m_sb = cp_sb[:, li * 8 + 4:li * 8 + 6]
            v_sb = cp_sb[:, li * 8 + 6:li * 8 + 8]
            sq = const.tile([128, NG], F32, tag=f"bn_sq{li}")
            nc.scalar.activation(out=sq, in_=v_sb,
                                 func=mybir.ActivationFunctionType.Sqrt,
                                 bias=eps_sb, scale=1.0)
            rs = const.tile([128, NG], F32, tag=f"bn_rs{li}")
            nc.vector.reciprocal(out=rs, in_=sq)
            s_sb = const.tile([128, NG], F32, tag=f"bn_s{li}")
            nc.vector.tensor_mul(s_sb, g_sb, rs)
            t_sb = const.tile([128, NG], F32, tag=f"bn_t{li}")
            nc.vector.tensor_mul(t_sb, m_sb, s_sb)
            h_sb = const.tile([128, NG], F32, tag=f"bn_h{li}")
            nc.vector.tensor_sub(h_sb, b_sb, t_sb)
            bn_sh[li] = (s_sb, h_sb)

        # ---- bank group DMAs in consumption order: the first six on the
        #      sync ring (gpsimd's queue is busy with pads until ~16us),
        #      the rest alternating gpsimd/sync ----
        wt = {}
        for li in range(2):
            for gg in range(NGRP):
                k = li * NGRP + gg
                t = wpool.tile([128, GCOLS], BF, tag="wt",
                               name=f"wt_{li}_{gg}")
                eng = nc.sync if (k < 6 or k % 2 == 1) else nc.gpsimd
                eng.dma_start(
                    out=t,
                    in_=bass.AP(tensor=w_d[li], offset=gg * GCOLS,
                                ap=[[NGRP * GCOLS, 128], [1, GCOLS]]))
                wt[(li, gg)] = t

        # y1p borders (gpsimd; only needed before conv2 reads)
        for b in range(BS):
            for g in range(NG):
                nc.gpsimd.memset(y1p[b][g][:, 0:PAD:33, :], 0.0)
                nc.gpsimd.memset(y1p[b][g][:, 1:33, 0:PAD:33], 0.0)

        # ---- single kT buffer for both layers (range-tracked deps let
        #      layer-1 groups overwrite regions as layer-0 finishes) ----
        kT = kpool.tile([128, BS, NG * NGRP * 128], BF, tag="kT",
                        name="kT")

        def comb_group(li, gg):
            t = wt[(li, gg)]
            mps = mpsum.tile([128, 1024], F32, tag="mps",
                             name=f"mps_{li}_{gg}")
            for k in range(16):
                nc.tensor.matmul(mps[:, k * 64:(k + 1) * 64],
                                 t[:, k * 128:(k + 1) * 128], r_mat,
                                 start=True, stop=True)
            # evict: psum col = ((cc*2+ig)*4 + b)*16 + o  ->  kT
            src4 = mps.rearrange("p (cc ig b o) -> p ig b cc o",
                                 cc=8, ig=2, b=BS, o=16)
            for ig in range(NG):
                d0 = ig * NGRP * 128 + gg * 128
                dst = kT[:, :, d0:d0 + 128].rearrange(
                    "p b (cc o) -> p b cc o", cc=8, o=16)
                nc.scalar.activation(
                    out=dst, in_=src4[:, ig],
                    func=mybir.ActivationFunctionType.Copy,
                    bias=0.0, scale=1.0)

        def kslice(ig, gg, b):
            c0 = ig * NGRP * 128 + gg * 128
            return kT[:, b, c0:c0 + 128]

        def epilogue(li, og, rh, b, ps, osb):
            s_sb, h_sb = bn_sh[li]
            psr = ps.rearrange("p (r c) -> p r c", r=16)
            if li == 0:
                nc.scalar.activation(
                    out=y1p[b][og][:, 1 + rh * 16:1 + rh * 16 + 16, 1:33],
                    in_=psr,
                    func=mybir.ActivationFunctionType.Relu,
                    bias=h_sb[:, og:og + 1],
                    scale=s_sb[:, og:og + 1])
            else:
                nc.vector.scalar_tensor_tensor(
                    out=psr, in0=psr, scalar=s_sb[:, og:og + 1],
                    in1=xp[b][og][:, 1 + rh * 16:1 + rh * 16 + 16, 1:33],
                    op0=mybir.AluOpType.mult,
                    op1=mybir.AluOpType.add)
                nc.scalar.activation(
                    out=osb[b][:, rh * 512:rh * 512 + 512].rearrange(
                        "p (r c) -> p r c", r=16),
                    in_=psr,
                    func=mybir.ActivationFunctionType.Relu,
                    bias=h_sb[:, og:og + 1], scale=1.0)
                if rh == 1:
                    dst = bass.AP(
                        tensor=out_d,
                        offset=(b * C + og * 128) * H * W,
                        ap=[[H * W, 128], [1, H * W]])
                    eng = nc.gpsimd if b % 2 == 0 else nc.sync
                    eng.dma_start(out=dst, in_=osb[b])

        def conv_mm(li, og, rh, gl, ig, b, ps, src_t):
            gg = og * 9 + gl
            dy, dx = divmod(gl, 3)
            t = gl * 2 + ig
            nc.tensor.matmul(
                ps[b], kslice(ig, gg, b),
                src_t[b][ig][:, rh * 16 + dy:rh * 16 + dy + 16, dx:dx + 32],
                start=(t == 0), stop=(t == 17))

        def rh_unit(li, og, rh, osb=None, self_comb=False, pre_comb=None,
                    b_major=False):
            src_t = xp if li == 0 else y1p
            ps = [cpsum.tile([128, 512], F32, tag="cps",
                             name=f"cps_{li}_{og}_{rh}_{b}")
                  for b in range(BS)]
            if b_major:
                for b in range(BS):
                    for gl in range(9):
                        for ig in range(NG):
                            conv_mm(li, og, rh, gl, ig, b, ps, src_t)
                    epilogue(li, og, rh, b, ps[b], osb)
                return
            if self_comb:
                comb_group(li, og * 9)
                comb_group(li, og * 9 + 1)
            for gl in range(9):
                for ig in range(NG):
                    for b in range(BS):
                        conv_mm(li, og, rh, gl, ig, b, ps, src_t)
                        if gl == 8 and ig == NG - 1:
                            epilogue(li, og, rh, b, ps[b], osb)
                if ig == NG - 1 and self_comb and gl + 2 <= 8:
                    comb_group(li, og * 9 + gl + 2)
                if pre_comb is not None:
                    comb_group(*pre_comb[gl])

        # phase schedule; comb for phase p+1 inside phase p's rh1 unit
        osb_all = {}
        for og in range(2):
            osb_all[og] = [opool.tile([128, 1024], BF, tag="osb",
                                      name=f"osb_{og}_{b}")
                           for b in range(BS)]
        rh_unit(0, 0, 0, self_comb=True)
        rh_unit(0, 0, 1, pre_comb=[(0, 9 + gl) for gl in range(9)])
        rh_unit(0, 1, 0)
        rh_unit(0, 1, 1, pre_comb=[(1, gl) for gl in range(9)])
        rh_unit(1, 0, 0, osb=osb_all[0])
        rh_unit(1, 0, 1, osb=osb_all[0],
                pre_comb=[(1, 9 + gl) for gl in range(9)])
        rh_unit(1, 1, 0, osb=osb_all[1])
        rh_unit(1, 1, 1, osb=osb_all[1], b_major=True)
    nc.compile()
    return nc


def _get_nc():
    if "nc" not in _BUILT:
        _BUILT["nc"] = build()
    return _BUILT["nc"]


def _host_transform_bank(w):
    """[E, O, I, 3, 3] f32 -> [p=(e,o16), ((og,dy,dx), cc, ig, i)] bf16."""
    import ml_dtypes
    wr = w.reshape(E, NG, 128, NG, 128, 3, 3)      # e og o_l ig i_l dy dx
    t = wr.transpose(1, 5, 6, 3, 0, 2, 4)          # og dy dx ig e o_l i_l
    t = t.reshape(NGRP, NG, E, 8, 16, 128)         # gg ig e cc o16 i
    t = t.transpose(0, 3, 1, 2, 4, 5)              # gg cc ig e o16 i
    t = t.reshape(NGRP, 8, NG, 128, 128)           # gg cc ig p i
    t = t.transpose(3, 0, 1, 2, 4)                 # p gg cc ig i
    t = t.reshape(128, NGRP * GCOLS)
    return np.ascontiguousarray(t.astype(ml_dtypes.bfloat16))


def _host_mask16():
    import ml_dtypes
    m = (np.arange(128)[:, None] % 16 == np.arange(16)[None, :])
    return m.astype(ml_dtypes.bfloat16)


def _host_irep():
    return (np.arange(128)[None, :] // 16 ==
            np.arange(E)[:, None]).astype(np.float32)


def _host_cpack(f):
    """[128, 32] f32: cols (li*4+{g,b,m,v})*NG+g = bn, 16+g*E+e = router_w."""
    cp = np.zeros((128, 32), np.float32)
    for li, names in enumerate((("g1", "b1", "m1", "v1"),
                                ("g2", "b2", "m2", "v2"))):
        for k, nm in enumerate(names):
            cp[:, li * 8 + k * 2:li * 8 + k * 2 + 2] = \
                f[nm].reshape(NG, 128).T
    for g in range(NG):
        cp[:, 16 + g * E:16 + (g + 1) * E] = \
            f["router_w"][:, g * 128:(g + 1) * 128].T
    return cp


def run(inputs, trace=False):
    import ml_dtypes
    nc = _get_nc()
    full = {k: np.ascontiguousarray(np.asarray(v, dtype=np.float32))
            for k, v in inputs.items()}
    full["w1t"] = _host_transform_bank(full.pop("w1"))
    full["w2t"] = _host_transform_bank(full.pop("w2"))
    full["mask16"] = _host_mask16()
    full["irep"] = _host_irep()
    full["cpack"] = _host_cpack(full)
    for k in ("g1", "b1", "m1", "v1", "g2", "b2", "m2", "v2", "router_w"):
        full.pop(k)
    xbf = np.ascontiguousarray(full.pop("x").astype(ml_dtypes.bfloat16))
    in_maps = []
    for j in range(NCORES):
        m = dict(full)
        m["x"] = np.ascontiguousarray(xbf[j * BS:(j + 1) * BS])
        in_maps.append(m)
    res = run_bass_kernel_spmd(nc, in_maps, core_ids=list(range(NCORES)),
                               trace=trace)
    out = np.concatenate([res.results[j]["out"] for j in range(NCORES)],
                         axis=0).astype(np.float32)
    return out, res


def kernel(**inputs) -> np.ndarray:
    out, _ = run(inputs, trace=False)
    return out
